# revision 20
# baseline (speedup 1.0000x reference)
"""Causal multi-head attention (QKV projection + softmax(QK^T)V) on 8 TRN2 NeuronCores.

Problem: x[4,2048,1024] @ W_qkv[1024,3072] + b_qkv -> 16-head causal attention -> [4,2048,1024].

Sharding: core i = (batch bi=i//2, head-group hg=i%2). Each core handles 1 batch x 8 heads,
fully data/tensor-parallel (no collectives). Host pre-arranges per-core inputs (all matmul
operands fp16; accumulation f32 in PSUM), repacked so every DMA reads multi-KB contiguous
runs per partition:
  - x^T as [128, stripe, k-chunk, 512]: one DMA per 512-token stripe, 8KB/partition
    contiguous; per-stripe SBUF tiles keep DMA-completion deps fine-grained.
  - wqk pair-major as [128, pair, k-chunk, 256] (pair p: Q cols then K cols, head-PAIR
    stacked 64+64 rows): pair-0 its own tile/DMA so the first QK matmul waits on 0.5 MB.
  - wv [128, k-chunk, 520]: V columns with per-head stride 65; col 65h+64 is a zero
    column, and the replicated bias tile bv has 1.0 there, so the "ones column" that
    makes the PV matmul accumulate softmax denominators (and b_v itself) ride the DVE
    PSUM->SBUF drain as a tensor_add -- no bias matmuls at all.
Device pipeline per core:
  QKV^T matmuls -> S^T = K Q^T per key-chunk with causal column trimming (two K=64
  row-tiled matmuls per chunk, one per head, into the two banks of one PSUM tile) ->
  one ScalarE Exp(scale=1/8) per chunk covering both heads [2,512], PSUM->SBUF fp16 =
  P^T -> causal tri-mask multiply on the 128x128 diagonal blocks only (DVE) -> PV
  matmuls accumulate [q, 64 cols + denominator] per q-block (both heads packed in one
  PSUM bank) -> reciprocal (DVE) * scale (DVE) epilogue -> DMA out per (pair, q-block).
Scheduling: blocks run PAIR-MAJOR (all 4 stripes of head-pair 0, then pair 1, ...).
t-major order skews the workload (early stripes are exp-light -> TensorE-bound first
half, ScalarE-bound endgame); pair-major gives every phase ~1/4 of the exp work against
~1/4 of the matmul work so both engines stay loaded throughout. QKV tiles are a
demand-ordered filler queue drained between S^T chunks (next block's tiles prefetched
via hooks, one V or PV pop per hook); each stripe's PV is deferred into the pair's next
block (inlined per-diagonal-chunk in the final block). V half-0 is produced during
pair-0's phase, half-1 prefetched during pair-1's. Input DMA: few big repacked
transfers on the sync ring ordered by first consumption (wqk pair-0, x stripes 0-1, wv,
x 2-3); constants + remaining wqk pairs ride the scalar HWDGE ring in parallel.
"""

import numpy as np

import concourse.bass as bass
import concourse.tile as tile
from concourse import bacc, mybir
from concourse import bass_utils

F16 = mybir.dt.float16
F32 = mybir.dt.float32

B, N, D = 4, 2048, 1024
H = 16  # global heads
HD = 64
HL = 8  # heads per core
N_CORES = 8
P = 128
NT = N // P  # 16 token tiles
KC = D // P  # 8 contraction chunks
VW = HL * (HD + 1)  # 520
VH = VW // 2  # 260

_cache = {}


def _build():
    nc = bacc.Bacc("TRN2", target_bir_lowering=False, debug=False)

    x_d = nc.dram_tensor("x", [P, 4, KC, 512], F16, kind="ExternalInput").ap()
    wqk_d = nc.dram_tensor("wqk", [P, 4, KC, 256], F16, kind="ExternalInput").ap()
    wv_d = nc.dram_tensor("wv", [P, KC, VW], F16, kind="ExternalInput").ap()
    bqk_d = nc.dram_tensor("bqk", [P, 8], F32, kind="ExternalInput").ap()
    bv_d = nc.dram_tensor("bv", [P, VW], F16, kind="ExternalInput").ap()
    tri_d = nc.dram_tensor("tri", [P, P], F16, kind="ExternalInput").ap()
    out_d = nc.dram_tensor("out", [N, HL * HD], F32, kind="ExternalOutput").ap()

    with tile.TileContext(nc) as tc:
        with (
            tc.tile_pool(name="const", bufs=1) as cpool,
            tc.tile_pool(name="pt", bufs=2) as ptpool,
            tc.tile_pool(name="opair", bufs=3) as oppool,
            tc.tile_pool(name="misc", bufs=6) as mpool,
            tc.tile_pool(name="ps_mm", bufs=1, space="PSUM") as ps_mm,
            tc.tile_pool(name="ps_s", bufs=3, space="PSUM") as ps_s,
            tc.tile_pool(name="ps_o", bufs=1, space="PSUM") as ps_o,
        ):
            # ---- constants / inputs to SBUF ----
            x_sb = [cpool.tile([P, KC, 512], F16, name=f"x{tt}_sb") for tt in range(4)]
            wqk0_sb = cpool.tile([P, KC, 256], F16, name="wqk0_sb")  # pair 0
            wqkR_sb = cpool.tile([P, KC, 768], F16, name="wqkR_sb")  # pairs 1-3
            wv_sb = cpool.tile([P, KC, VW], F16, name="wv_sb")
            bqk_sb = cpool.tile([P, 8], F32, name="bqk_sb")
            bv_sb = cpool.tile([P, VW], F16, name="bv_sb")  # b_v (+ones col) replicated
            tri_sb = cpool.tile([P, P], F16, name="tri_sb")
            qt_sb = cpool.tile([P, 4, N], F16, name="qt_sb")  # Q^T pair-stacked
            kt_sb = cpool.tile([P, 4, N], F16, name="kt_sb")  # K^T pair-stacked
            v_sb = cpool.tile([P, NT, VW], F16, name="v_sb")
            # P^T staging, STATIC per stripe (plane = 2*chunk+hh), reused across
            # pairs: pair p's stripe-t chains drain during pair p's own phase, a full
            # phase before pair p+1 rewrites pt_t -- so the deferred PV chains get a
            # whole phase of slack instead of a 2-block pool deadline.
            pt_t = [
                cpool.tile([P, 2 * (4 * t + 4), 512], F16, name=f"pt{t}_sb")
                for t in range(4)
            ]

            # Preload the exp table set during the DMA fill, so the first real
            # softmax exp doesn't pay ACT_TABLE_LOAD.
            warm = mpool.tile([1, 8], F32, tag="warm", name="warm")
            nc.vector.memset(warm[:], 0.0)
            nc.scalar.activation(warm[:], warm[:], mybir.ActivationFunctionType.Exp)
            # Input DMA ordered by first consumption under the pair-major schedule.
            nc.sync.dma_start(wqk0_sb[:], wqk_d[:, 0, :, :])  # 0.5 MB
            nc.sync.dma_start(x_sb[0][:], x_d[:, 0, :, :])  # 1 MB
            nc.sync.dma_start(x_sb[1][:], x_d[:, 1, :, :])
            nc.sync.dma_start(wv_sb[:], wv_d)  # 1 MB
            nc.sync.dma_start(x_sb[2][:], x_d[:, 2, :, :])
            nc.sync.dma_start(x_sb[3][:], x_d[:, 3, :, :])
            nc.scalar.dma_start(bqk_sb[:], bqk_d)
            nc.scalar.dma_start(tri_sb[:], tri_d)
            nc.scalar.dma_start(bv_sb[:], bv_d)
            nc.scalar.dma_start(wqkR_sb[:, :, 0:256], wqk_d[:, 1, :, :])
            nc.scalar.dma_start(wqkR_sb[:, :, 256:512], wqk_d[:, 2, :, :])
            nc.scalar.dma_start(wqkR_sb[:, :, 512:768], wqk_d[:, 3, :, :])

            done_qk = set()
            done_v = set()

            def wqk_slice(c, k):
                pr = c % 4
                off = 0 if c < 4 else 128
                if pr == 0:
                    return wqk0_sb[:, k, off : off + P]
                base = 256 * (pr - 1) + off
                return wqkR_sb[:, k, base : base + P]

            def emit_qk(c, tt):
                """QKV^T matmul tile for col-chunk c, token stripe tt."""
                if (c, tt) in done_qk:
                    return
                done_qk.add((c, tt))
                pr = c % 4
                pq = ps_mm.tile([P, 512], F32, tag="mm", name=f"pq_{c}_{tt}")
                for k in range(KC):
                    nc.tensor.matmul(
                        pq[:],
                        lhsT=wqk_slice(c, k),
                        rhs=x_sb[tt][:, k, :],
                        start=(k == 0),
                        stop=(k == KC - 1),
                    )
                dst = qt_sb if c < 4 else kt_sb
                nc.vector.tensor_scalar_add(
                    dst[:, pr, tt * 512 : (tt + 1) * 512], pq[:], bqk_sb[:, c : c + 1]
                )

            def emit_v(j, half):
                """V (augmented) for token tile j, half (260 cols each)."""
                if (j, half) in done_v:
                    return
                done_v.add((j, half))
                pv = ps_mm.tile([P, VH], F32, tag="mm", name=f"pv_{j}_{half}")
                for k in range(KC):
                    nc.tensor.matmul(
                        pv[:],
                        lhsT=x_sb[j // 4][:, k, (j % 4) * P : (j % 4 + 1) * P],
                        rhs=wv_sb[:, k, half * VH : (half + 1) * VH],
                        start=(k == 0),
                        stop=(k == KC - 1),
                    )
                # bias (and the denominator ones-column) ride the PSUM->SBUF drain
                nc.vector.tensor_add(
                    v_sb[:, j, half * VH : (half + 1) * VH],
                    pv[:],
                    bv_sb[:, half * VH : (half + 1) * VH],
                )

            state = {"po_n": 0}
            # PV accumulators: both heads AND the ping-pong parity packed into ONE
            # PSUM bank (2*2*65*4 = 1040B < 2KB) so ps_s can triple-buffer.
            po_all = ps_o.tile([P, 2, 2, 65], F32, name="po_all")

            def emit_pv_half(p, t, pt, r, hh, ctx):
                """One head's PV chain for q-block i = 4t+r; epilogue+DMA after hh=1."""
                i = 4 * t + r
                po = po_all[:, ctx["par"], :, :]
                if hh == 0:
                    ctx["opair"] = oppool.tile([P, P], F32, tag="op", name=f"op_{p}_{i}")
                for j in range(i + 1):
                    nc.tensor.matmul(
                        po[:, hh, :],
                        lhsT=pt[:, 2 * j + hh, r * P : (r + 1) * P],
                        rhs=v_sb[:, j, 65 * (2 * p + hh) : 65 * (2 * p + hh) + 65],
                        start=(j == 0),
                        stop=(j == i),
                    )
                if hh == 0:
                    return
                opair = ctx["opair"]
                rc = mpool.tile([P, 2], F32, tag="rc", name=f"rc_{p}_{i}")
                nc.vector.reciprocal(rc[:], po[:, :, 64])
                for h2 in (0, 1):
                    nc.vector.tensor_scalar_mul(
                        opair[:, 64 * h2 : 64 * h2 + 64], po[:, h2, 0:64], rc[:, h2 : h2 + 1]
                    )
                nc.sync.dma_start(out_d[i * P : (i + 1) * P, p * P : (p + 1) * P], opair[:])

            def emit_pv(p, t, pt, r):
                ctx = {"par": state["po_n"] % 2}
                state["po_n"] += 1
                emit_pv_half(p, t, pt, r, 0, ctx)
                emit_pv_half(p, t, pt, r, 1, ctx)

            # Prologue: just the first QK tiles so S^T (pair 0, stripe 0) starts ASAP.
            emit_qk(0, 0)
            emit_qk(4, 0)

            pv_queue = []
            # PAIR-MAJOR: all four stripes of pair 0, then pair 1, ...
            blocks = [(p, t) for p in range(4) for t in range(4)]
            for n, (p, t) in enumerate(blocks):
                    last = n == len(blocks) - 1
                    for tt in range(t + 1):
                        emit_qk(p, tt)
                        emit_qk(4 + p, tt)
                    # guard: previous pair's stripe-t chains read pt_t -- finish them
                    # before this block's exps rewrite it (normally already drained
                    # by the one-pop-per-hook pacing)
                    for ch in [c for c in pv_queue if c[1] == t]:
                        pv_queue.remove(ch)
                        emit_pv(*ch)
                    # tiles the NEXT block's S^T will need, prefetched via hooks
                    nxt_qk = []
                    if n + 1 < len(blocks):
                        pn, tn = blocks[n + 1]
                        nxt_qk = [
                            (c, tt)
                            for tt in range(tn + 1)
                            for c in (pn, 4 + pn)
                            if (c, tt) not in done_qk
                        ]
                    # pt layout: [128, plane=2*chunk+hh, 512]
                    pt = pt_t[t]
                    # V tiles: pair 0's phase produces half 0 (its own PV demand),
                    # pair 1's phase prefetches half 1 for pairs 2-3.
                    if p == 0:
                        vpend = [(j, 0) for j in range(4 * t, 4 * t + 4) if (j, 0) not in done_v]
                    elif p == 1:
                        vpend = [(j, 1) for j in range(4 * t, 4 * t + 4) if (j, 1) not in done_v]
                    else:
                        vpend = [
                            (j, p // 2)
                            for j in range(4 * t, 4 * t + 4)
                            if (j, p // 2) not in done_v
                        ]

                    def chunk_hooks(vpend=vpend, nxt_qk=nxt_qk):
                        if pv_queue:
                            emit_pv(*pv_queue.pop(0))
                        elif vpend:
                            emit_v(*vpend.pop(0))
                        if nxt_qk:
                            emit_qk(*nxt_qk.pop(0))

                    # S^T + exp per key-chunk: the pair's two heads run as K=64
                    # row-tiled matmuls (array row-groups 0-1 / 2-3, adjacent PSUM
                    # banks), then one Exp covers both heads. Diagonal chunks only
                    # compute/exp the causal-valid column suffix.
                    for j in range(4 * t + 4):
                        psC = ps_s.tile([P, 2, 512], F32, tag="s", name=f"ps_{p}_{t}_{j}")
                        q0 = 128 * (j - 4 * t) if j >= 4 * t else 0
                        for hh in (0, 1):
                            nc.tensor.matmul(
                                psC[:, hh, q0:512],
                                lhsT=kt_sb[64 * hh : 64 * hh + 64, p, j * P : (j + 1) * P],
                                rhs=qt_sb[
                                    64 * hh : 64 * hh + 64,
                                    p,
                                    t * 512 + q0 : (t + 1) * 512,
                                ],
                                start=True,
                                stop=True,
                            )
                        nc.scalar.activation(
                            pt[:, 2 * j : 2 * j + 2, q0:512],
                            psC[:, :, q0:512],
                            mybir.ActivationFunctionType.Exp,
                            scale=0.125,
                        )
                        chunk_hooks()
                        if last and j >= 4 * t:
                            # final block: mask + PV inline per diagonal chunk so the
                            # tail doesn't serialize after the last exp
                            r = j - 4 * t
                            for hh in (0, 1):
                                blk = pt[:, 2 * j + hh, r * P : (r + 1) * P]
                                nc.vector.tensor_mul(blk, blk, tri_sb[:])
                            emit_pv(p, t, pt, r)
                    if last:
                        continue
                    # causal mask on diagonal 128x128 blocks (DVE: fast and it has
                    # slack; next block's PV pops need these early)
                    for hh in (0, 1):
                        for r in range(4):
                            j = 4 * t + r
                            blk = pt[:, 2 * j + hh, r * P : (r + 1) * P]
                            nc.vector.tensor_mul(blk, blk, tri_sb[:])
                    # V tiles this stripe's PV will need (PV runs during the next
                    # block; guard: force any stragglers now)
                    for j in range(4 * t + 4):
                        emit_v(j, p // 2)
                    pv_queue = [(p, t, pt, r) for r in range(4)]
            while pv_queue:
                emit_pv(*pv_queue.pop(0))
            # flush any unprefetched QKV (normally none)
            for tt in range(4):
                for c in range(8):
                    emit_qk(c, tt)
                for j in range(4 * tt, 4 * tt + 4):
                    emit_v(j, 0)
                    emit_v(j, 1)

    nc.compile()
    return nc


def get_nc():
    if "nc" not in _cache:
        _cache["nc"] = _build()
    return _cache["nc"]


def _prep_core_inputs(x, W, b, bi, hg):
    h0 = hg * HL
    Wq = W[:, 0:D].reshape(D, H, HD)
    Wk = W[:, D : 2 * D].reshape(D, H, HD)
    Wv = W[:, 2 * D :].reshape(D, H, HD)
    bq = b[0:D].reshape(H, HD)
    bk = b[D : 2 * D].reshape(H, HD)
    bv = b[2 * D :].reshape(H, HD)

    # pair-major: pair p occupies cols [256p, 256p+256) as [Q pair | K pair]
    wqk = np.empty((D, 1024), np.float32)
    bqk = np.empty((P, 8), np.float32)
    for c in range(4):
        for half in range(2):
            h = h0 + 2 * c + half
            sl = slice(256 * c + half * HD, 256 * c + half * HD + HD)
            wqk[:, sl] = Wq[:, h]
            bqk[half * HD : (half + 1) * HD, c] = bq[h]
            sl = slice(256 * c + P + half * HD, 256 * c + P + half * HD + HD)
            wqk[:, sl] = Wk[:, h]
            bqk[half * HD : (half + 1) * HD, 4 + c] = bk[h]

    wv_aug = np.zeros((D, VW), np.float32)
    bv_aug = np.zeros((VW,), np.float32)
    for hl in range(HL):
        wv_aug[:, 65 * hl : 65 * hl + HD] = Wv[:, h0 + hl]
        bv_aug[65 * hl : 65 * hl + HD] = bv[h0 + hl]
        bv_aug[65 * hl + HD] = 1.0

    tri = np.triu(np.ones((P, P), np.float32))  # tri[k, q] = 1 where q >= k

    # Repack for contiguous-per-partition DMA runs:
    xT = np.ascontiguousarray(x[bi].astype(np.float16).T)
    xh = np.ascontiguousarray(xT.reshape(KC, P, 4, 512).transpose(1, 2, 0, 3))
    wqkh = np.ascontiguousarray(
        wqk.astype(np.float16).reshape(KC, P, 4, 256).transpose(1, 2, 0, 3)
    )
    wvh = np.ascontiguousarray(
        wv_aug.astype(np.float16).reshape(KC, P, VW).transpose(1, 0, 2)
    )

    return {
        "x": xh,
        "wqk": wqkh,
        "wv": wvh,
        "bqk": bqk,
        "bv": np.broadcast_to(bv_aug.astype(np.float16), (P, VW)).copy(),
        "tri": tri.astype(np.float16),
    }


def make_in_maps(x, W_qkv, b_qkv):
    x = np.asarray(x, dtype=np.float32)
    W = np.asarray(W_qkv, dtype=np.float32)
    b = np.asarray(b_qkv, dtype=np.float32)
    return [_prep_core_inputs(x, W, b, i // 2, i % 2) for i in range(N_CORES)]


def assemble(results):
    out = np.empty((B, N, D), np.float32)
    for i in range(N_CORES):
        bi, hg = i // 2, i % 2
        out[bi, :, hg * 512 : (hg + 1) * 512] = results[i]["out"]
    return out


def run(x, W_qkv, b_qkv, trace=False, tmpdir=None):
    nc = get_nc()
    in_maps = make_in_maps(x, W_qkv, b_qkv)
    res = bass_utils.run_bass_kernel_spmd(
        nc, in_maps, core_ids=list(range(N_CORES)), trace=trace, tmpdir=tmpdir
    )
    return assemble(res.results), res


def kernel(x, W_qkv, b_qkv):
    out, _ = run(x, W_qkv, b_qkv)
    return out


# revision 23
# speedup vs baseline: 1.2366x; 1.2366x over previous
"""Causal multi-head attention (QKV projection + softmax(QK^T)V) on 8 TRN2 NeuronCores.

Problem: x[4,2048,1024] @ W_qkv[1024,3072] + b_qkv -> 16-head causal attention -> [4,2048,1024].

Sharding: core i = (batch bi=i//2, head-group hg=i%2). Each core handles 1 batch x 8 heads,
fully data/tensor-parallel (no collectives). Host pre-arranges per-core inputs (all matmul
operands fp16; accumulation f32 in PSUM), repacked so every DMA reads multi-KB contiguous
runs per partition:
  - x^T as [128, stripe, k-chunk, 512]: one DMA per 512-token stripe, 8KB/partition
    contiguous; per-stripe SBUF tiles keep DMA-completion deps fine-grained.
  - wqk pair-major as [128, pair, k-chunk, 256] (pair p: Q cols then K cols, head-PAIR
    stacked 64+64 rows): pair-0 its own tile/DMA so the first QK matmul waits on 0.5 MB.
  - wv [128, k-chunk, 520]: V columns with per-head stride 65; col 65h+64 is a zero
    column, and the replicated bias tile bv has 1.0 there, so the "ones column" that
    makes the PV matmul accumulate softmax denominators (and b_v itself) ride the DVE
    PSUM->SBUF drain as a tensor_add -- no bias matmuls at all.
Device pipeline per core:
  QKV^T matmuls -> S^T = K Q^T per key-chunk with causal column trimming (two K=64
  row-tiled matmuls per chunk, one per head, into the two banks of one PSUM tile) ->
  one ScalarE Exp(scale=1/8) per chunk covering both heads [2,512], PSUM->SBUF fp16 =
  P^T -> causal tri-mask multiply on the 128x128 diagonal blocks only (DVE) -> PV
  matmuls accumulate [q, 64 cols + denominator] per q-block (both heads packed in one
  PSUM bank) -> reciprocal (DVE) * scale (DVE) epilogue -> DMA out per (pair, q-block).
Scheduling: blocks run PAIR-MAJOR (all 4 stripes of head-pair 0, then pair 1, ...).
t-major order skews the workload (early stripes are exp-light -> TensorE-bound first
half, ScalarE-bound endgame); pair-major gives every phase ~1/4 of the exp work against
~1/4 of the matmul work so both engines stay loaded throughout. QKV tiles are a
demand-ordered filler queue drained between S^T chunks (next block's tiles prefetched
via hooks, one V or PV pop per hook); each stripe's PV is deferred into the pair's next
block (inlined per-diagonal-chunk in the final block). V half-0 is produced during
pair-0's phase, half-1 prefetched during pair-1's. Input DMA: few big repacked
transfers on the sync ring ordered by first consumption (wqk pair-0, x stripes 0-1, wv,
x 2-3); constants + remaining wqk pairs ride the scalar HWDGE ring in parallel.
"""

import numpy as np

import concourse.bass as bass
import concourse.tile as tile
from concourse import bacc, mybir
from concourse import bass_utils

F16 = mybir.dt.float16
F32 = mybir.dt.float32

B, N, D = 4, 2048, 1024
H = 16  # global heads
HD = 64
HL = 8  # heads per core
N_CORES = 8
P = 128
NT = N // P  # 16 token tiles
KC = D // P  # 8 contraction chunks
VW = HL * (HD + 1)  # 520
VH = VW // 2  # 260

_cache = {}


def _build():
    nc = bacc.Bacc("TRN2", target_bir_lowering=False, debug=False)

    x_d = nc.dram_tensor("x", [P, 4, KC, 512], F16, kind="ExternalInput").ap()
    wqk_d = nc.dram_tensor("wqk", [P, 4, KC, 256], F16, kind="ExternalInput").ap()
    wv_d = nc.dram_tensor("wv", [P, KC, VW], F16, kind="ExternalInput").ap()
    bqk_d = nc.dram_tensor("bqk", [P, 8], F32, kind="ExternalInput").ap()
    bv_d = nc.dram_tensor("bv", [P, VW], F16, kind="ExternalInput").ap()
    tri_d = nc.dram_tensor("tri", [P, P], F16, kind="ExternalInput").ap()
    out_d = nc.dram_tensor("out", [N, HL * HD], F32, kind="ExternalOutput").ap()

    with tile.TileContext(nc) as tc:
        with (
            tc.tile_pool(name="const", bufs=1) as cpool,
            tc.tile_pool(name="pt", bufs=2) as ptpool,
            tc.tile_pool(name="opair", bufs=3) as oppool,
            tc.tile_pool(name="misc", bufs=6) as mpool,
            tc.tile_pool(name="ps_mm", bufs=2, space="PSUM") as ps_mm,
            tc.tile_pool(name="ps_s", bufs=2, space="PSUM") as ps_s,
            tc.tile_pool(name="ps_o", bufs=2, space="PSUM") as ps_o,
        ):
            # ---- constants / inputs to SBUF ----
            x_sb = [cpool.tile([P, KC, 512], F16, name=f"x{tt}_sb") for tt in range(4)]
            wqk0_sb = cpool.tile([P, KC, 256], F16, name="wqk0_sb")  # pair 0
            wqkR_sb = cpool.tile([P, KC, 768], F16, name="wqkR_sb")  # pairs 1-3
            wv_sb = cpool.tile([P, KC, VW], F16, name="wv_sb")
            bqk_sb = cpool.tile([P, 8], F32, name="bqk_sb")
            bv_sb = cpool.tile([P, VW], F16, name="bv_sb")  # b_v (+ones col) replicated
            tri_sb = cpool.tile([P, P], F16, name="tri_sb")
            qt_sb = cpool.tile([P, 4, N], F16, name="qt_sb")  # Q^T pair-stacked
            kt_sb = cpool.tile([P, 4, N], F16, name="kt_sb")  # K^T pair-stacked
            v_sb = cpool.tile([P, NT, VW], F16, name="v_sb")
            # P^T staging, STATIC per stripe (plane = 2*chunk+hh), reused across
            # pairs: pair p's stripe-t chains drain during pair p's own phase, a full
            # phase before pair p+1 rewrites pt_t -- so the deferred PV chains get a
            # whole phase of slack instead of a 2-block pool deadline.
            pt_t = [
                cpool.tile([P, 2 * (4 * t + 4), 512], F16, name=f"pt{t}_sb")
                for t in range(4)
            ]

            # Preload the exp table set during the DMA fill, so the first real
            # softmax exp doesn't pay ACT_TABLE_LOAD.
            warm = mpool.tile([1, 8], F32, tag="warm", name="warm")
            nc.vector.memset(warm[:], 0.0)
            nc.scalar.activation(warm[:], warm[:], mybir.ActivationFunctionType.Exp)
            # Input DMA ordered by first consumption under the pair-major schedule.
            nc.sync.dma_start(wqk0_sb[:], wqk_d[:, 0, :, :])  # 0.5 MB
            nc.sync.dma_start(x_sb[0][:], x_d[:, 0, :, :])  # 1 MB
            nc.sync.dma_start(x_sb[1][:], x_d[:, 1, :, :])
            nc.sync.dma_start(wv_sb[:], wv_d)  # 1 MB
            nc.sync.dma_start(x_sb[2][:], x_d[:, 2, :, :])
            nc.sync.dma_start(x_sb[3][:], x_d[:, 3, :, :])
            nc.scalar.dma_start(bqk_sb[:], bqk_d)
            nc.scalar.dma_start(tri_sb[:], tri_d)
            nc.scalar.dma_start(bv_sb[:], bv_d)
            nc.scalar.dma_start(wqkR_sb[:, :, 0:256], wqk_d[:, 1, :, :])
            nc.scalar.dma_start(wqkR_sb[:, :, 256:512], wqk_d[:, 2, :, :])
            nc.scalar.dma_start(wqkR_sb[:, :, 512:768], wqk_d[:, 3, :, :])

            done_qk = set()
            done_v = set()

            def wqk_slice(c, k):
                pr = c % 4
                off = 0 if c < 4 else 128
                if pr == 0:
                    return wqk0_sb[:, k, off : off + P]
                base = 256 * (pr - 1) + off
                return wqkR_sb[:, k, base : base + P]

            def emit_qk(c, tt):
                """QKV^T matmul tile for col-chunk c, token stripe tt."""
                if (c, tt) in done_qk:
                    return
                done_qk.add((c, tt))
                pr = c % 4
                pq = ps_mm.tile([P, 512], F32, tag="mm", name=f"pq_{c}_{tt}")
                for k in range(KC):
                    nc.tensor.matmul(
                        pq[:],
                        lhsT=wqk_slice(c, k),
                        rhs=x_sb[tt][:, k, :],
                        start=(k == 0),
                        stop=(k == KC - 1),
                    )
                dst = qt_sb if c < 4 else kt_sb
                nc.vector.tensor_scalar_add(
                    dst[:, pr, tt * 512 : (tt + 1) * 512], pq[:], bqk_sb[:, c : c + 1]
                )

            def emit_v(j, half):
                """V (augmented) for token tile j, half (260 cols each)."""
                if (j, half) in done_v:
                    return
                done_v.add((j, half))
                pv = ps_mm.tile([P, VH], F32, tag="mm", name=f"pv_{j}_{half}")
                for k in range(KC):
                    nc.tensor.matmul(
                        pv[:],
                        lhsT=x_sb[j // 4][:, k, (j % 4) * P : (j % 4 + 1) * P],
                        rhs=wv_sb[:, k, half * VH : (half + 1) * VH],
                        start=(k == 0),
                        stop=(k == KC - 1),
                    )
                # bias (and the denominator ones-column) ride the PSUM->SBUF drain
                nc.vector.tensor_add(
                    v_sb[:, j, half * VH : (half + 1) * VH],
                    pv[:],
                    bv_sb[:, half * VH : (half + 1) * VH],
                )

            state = {}

            def emit_pv_half(p, t, pt, r, hh, ctx):
                """One head's PV chain for q-block i = 4t+r; epilogue+DMA after hh=1.
                Both heads' accumulators share one PSUM bank ([128, 2, 65])."""
                i = 4 * t + r
                if hh == 0:
                    ctx["opair"] = oppool.tile([P, P], F32, tag="op", name=f"op_{p}_{i}")
                    ctx["po"] = po = ps_o.tile([P, 2, 65], F32, tag="o", name=f"po_{p}_{i}")
                else:
                    po = ctx["po"]
                for j in range(i + 1):
                    nc.tensor.matmul(
                        po[:, hh, :],
                        lhsT=pt[:, 2 * j + hh, r * P : (r + 1) * P],
                        rhs=v_sb[:, j, 65 * (2 * p + hh) : 65 * (2 * p + hh) + 65],
                        start=(j == 0),
                        stop=(j == i),
                    )
                if hh == 0:
                    return
                opair = ctx["opair"]
                rc = mpool.tile([P, 2], F32, tag="rc", name=f"rc_{p}_{i}")
                nc.vector.reciprocal(rc[:], po[:, :, 64])
                for h2 in (0, 1):
                    nc.vector.tensor_scalar_mul(
                        opair[:, 64 * h2 : 64 * h2 + 64], po[:, h2, 0:64], rc[:, h2 : h2 + 1]
                    )
                nc.sync.dma_start(out_d[i * P : (i + 1) * P, p * P : (p + 1) * P], opair[:])

            def emit_pv(p, t, pt, r):
                ctx = {}
                emit_pv_half(p, t, pt, r, 0, ctx)
                emit_pv_half(p, t, pt, r, 1, ctx)

            # Prologue: just the first QK tiles so S^T (pair 0, stripe 0) starts ASAP.
            emit_qk(0, 0)
            emit_qk(4, 0)

            pv_queue = []
            # PAIR-MAJOR: all four stripes of pair 0, then pair 1, ...
            blocks = [(p, t) for p in range(4) for t in range(4)]
            for n, (p, t) in enumerate(blocks):
                    last = n == len(blocks) - 1
                    for tt in range(t + 1):
                        emit_qk(p, tt)
                        emit_qk(4 + p, tt)
                    # guard: previous pair's stripe-t chains read pt_t -- finish them
                    # before this block's exps rewrite it (normally already drained
                    # by the one-pop-per-hook pacing)
                    for ch in [c for c in pv_queue if c[1] == t]:
                        pv_queue.remove(ch)
                        emit_pv(*ch)
                    # tiles the NEXT block's S^T will need, prefetched via hooks
                    nxt_qk = []
                    if n + 1 < len(blocks):
                        pn, tn = blocks[n + 1]
                        nxt_qk = [
                            (c, tt)
                            for tt in range(tn + 1)
                            for c in (pn, 4 + pn)
                            if (c, tt) not in done_qk
                        ]
                    # pt layout: [128, plane=2*chunk+hh, 512]
                    pt = pt_t[t]
                    # V tiles: pair 0's phase produces half 0 (its own PV demand),
                    # pair 1's phase prefetches half 1 for pairs 2-3.
                    if p == 0:
                        vpend = [(j, 0) for j in range(4 * t, 4 * t + 4) if (j, 0) not in done_v]
                    elif p == 1:
                        vpend = [(j, 1) for j in range(4 * t, 4 * t + 4) if (j, 1) not in done_v]
                    else:
                        vpend = [
                            (j, p // 2)
                            for j in range(4 * t, 4 * t + 4)
                            if (j, p // 2) not in done_v
                        ]

                    def chunk_hooks(hk, vpend=vpend, nxt_qk=nxt_qk):
                        # In the short t=0 blocks, pop PV chains only on alternate
                        # hooks: the previous pair's long stripe-3 chains then spread
                        # into the next (8-hook) block instead of bunching here and
                        # starving the exp stream at pair boundaries.
                        if pv_queue and (t > 0 or hk % 2 == 1 or not (vpend or nxt_qk)):
                            emit_pv(*pv_queue.pop(0))
                        elif vpend:
                            emit_v(*vpend.pop(0))
                        if nxt_qk:
                            emit_qk(*nxt_qk.pop(0))

                    # S^T + exp per key-chunk: the pair's two heads run as K=64
                    # row-tiled matmuls (array row-groups 0-1 / 2-3, adjacent PSUM
                    # banks), then one Exp covers both heads. Diagonal chunks only
                    # compute/exp the causal-valid column suffix.
                    for j in range(4 * t + 4):
                        psC = ps_s.tile([P, 2, 512], F32, tag="s", name=f"ps_{p}_{t}_{j}")
                        q0 = 128 * (j - 4 * t) if j >= 4 * t else 0
                        for hh in (0, 1):
                            nc.tensor.matmul(
                                psC[:, hh, q0:512],
                                lhsT=kt_sb[64 * hh : 64 * hh + 64, p, j * P : (j + 1) * P],
                                rhs=qt_sb[
                                    64 * hh : 64 * hh + 64,
                                    p,
                                    t * 512 + q0 : (t + 1) * 512,
                                ],
                                start=True,
                                stop=True,
                            )
                        nc.scalar.activation(
                            pt[:, 2 * j : 2 * j + 2, q0:512],
                            psC[:, :, q0:512],
                            mybir.ActivationFunctionType.Exp,
                            scale=0.125,
                        )
                        chunk_hooks(j)
                        if last and j >= 4 * t:
                            # final block: mask + PV inline per diagonal chunk so the
                            # tail doesn't serialize after the last exp
                            r = j - 4 * t
                            for hh in (0, 1):
                                blk = pt[:, 2 * j + hh, r * P : (r + 1) * P]
                                nc.vector.tensor_mul(blk, blk, tri_sb[:])
                            emit_pv(p, t, pt, r)
                    if last:
                        continue
                    # causal mask on diagonal 128x128 blocks (DVE: fast and it has
                    # slack; next block's PV pops need these early)
                    for hh in (0, 1):
                        for r in range(4):
                            j = 4 * t + r
                            blk = pt[:, 2 * j + hh, r * P : (r + 1) * P]
                            nc.vector.tensor_mul(blk, blk, tri_sb[:])
                    # V tiles this stripe's PV will need (PV runs during the next
                    # block; guard: force any stragglers now)
                    for j in range(4 * t + 4):
                        emit_v(j, p // 2)
                    pv_queue.extend((p, t, pt, r) for r in range(4))
            while pv_queue:
                emit_pv(*pv_queue.pop(0))
            # flush any unprefetched QKV (normally none)
            for tt in range(4):
                for c in range(8):
                    emit_qk(c, tt)
                for j in range(4 * tt, 4 * tt + 4):
                    emit_v(j, 0)
                    emit_v(j, 1)

    nc.compile()
    return nc


def get_nc():
    if "nc" not in _cache:
        _cache["nc"] = _build()
    return _cache["nc"]


def _prep_core_inputs(x, W, b, bi, hg):
    h0 = hg * HL
    Wq = W[:, 0:D].reshape(D, H, HD)
    Wk = W[:, D : 2 * D].reshape(D, H, HD)
    Wv = W[:, 2 * D :].reshape(D, H, HD)
    bq = b[0:D].reshape(H, HD)
    bk = b[D : 2 * D].reshape(H, HD)
    bv = b[2 * D :].reshape(H, HD)

    # pair-major: pair p occupies cols [256p, 256p+256) as [Q pair | K pair]
    wqk = np.empty((D, 1024), np.float32)
    bqk = np.empty((P, 8), np.float32)
    for c in range(4):
        for half in range(2):
            h = h0 + 2 * c + half
            sl = slice(256 * c + half * HD, 256 * c + half * HD + HD)
            wqk[:, sl] = Wq[:, h]
            bqk[half * HD : (half + 1) * HD, c] = bq[h]
            sl = slice(256 * c + P + half * HD, 256 * c + P + half * HD + HD)
            wqk[:, sl] = Wk[:, h]
            bqk[half * HD : (half + 1) * HD, 4 + c] = bk[h]

    wv_aug = np.zeros((D, VW), np.float32)
    bv_aug = np.zeros((VW,), np.float32)
    for hl in range(HL):
        wv_aug[:, 65 * hl : 65 * hl + HD] = Wv[:, h0 + hl]
        bv_aug[65 * hl : 65 * hl + HD] = bv[h0 + hl]
        bv_aug[65 * hl + HD] = 1.0

    tri = np.triu(np.ones((P, P), np.float32))  # tri[k, q] = 1 where q >= k

    # Repack for contiguous-per-partition DMA runs:
    xT = np.ascontiguousarray(x[bi].astype(np.float16).T)
    xh = np.ascontiguousarray(xT.reshape(KC, P, 4, 512).transpose(1, 2, 0, 3))
    wqkh = np.ascontiguousarray(
        wqk.astype(np.float16).reshape(KC, P, 4, 256).transpose(1, 2, 0, 3)
    )
    wvh = np.ascontiguousarray(
        wv_aug.astype(np.float16).reshape(KC, P, VW).transpose(1, 0, 2)
    )

    return {
        "x": xh,
        "wqk": wqkh,
        "wv": wvh,
        "bqk": bqk,
        "bv": np.broadcast_to(bv_aug.astype(np.float16), (P, VW)).copy(),
        "tri": tri.astype(np.float16),
    }


def make_in_maps(x, W_qkv, b_qkv):
    x = np.asarray(x, dtype=np.float32)
    W = np.asarray(W_qkv, dtype=np.float32)
    b = np.asarray(b_qkv, dtype=np.float32)
    return [_prep_core_inputs(x, W, b, i // 2, i % 2) for i in range(N_CORES)]


def assemble(results):
    out = np.empty((B, N, D), np.float32)
    for i in range(N_CORES):
        bi, hg = i // 2, i % 2
        out[bi, :, hg * 512 : (hg + 1) * 512] = results[i]["out"]
    return out


def run(x, W_qkv, b_qkv, trace=False, tmpdir=None):
    nc = get_nc()
    in_maps = make_in_maps(x, W_qkv, b_qkv)
    res = bass_utils.run_bass_kernel_spmd(
        nc, in_maps, core_ids=list(range(N_CORES)), trace=trace, tmpdir=tmpdir
    )
    return assemble(res.results), res


def kernel(x, W_qkv, b_qkv):
    out, _ = run(x, W_qkv, b_qkv)
    return out


# revision 24
# speedup vs baseline: 1.2433x; 1.0054x over previous
"""Causal multi-head attention (QKV projection + softmax(QK^T)V) on 8 TRN2 NeuronCores.

Problem: x[4,2048,1024] @ W_qkv[1024,3072] + b_qkv -> 16-head causal attention -> [4,2048,1024].

Sharding: core i = (batch bi=i//2, head-group hg=i%2). Each core handles 1 batch x 8 heads,
fully data/tensor-parallel (no collectives). Host pre-arranges per-core inputs (all matmul
operands fp16; accumulation f32 in PSUM), repacked so every DMA reads multi-KB contiguous
runs per partition:
  - x^T as [128, stripe, k-chunk, 512]: one DMA per 512-token stripe, 8KB/partition
    contiguous; per-stripe SBUF tiles keep DMA-completion deps fine-grained.
  - wqk pair-major as [128, pair, k-chunk, 256] (pair p: Q cols then K cols, head-PAIR
    stacked 64+64 rows): pair-0 its own tile/DMA so the first QK matmul waits on 0.5 MB.
  - wv [128, k-chunk, 520]: V columns with per-head stride 65; col 65h+64 is a zero
    column, and the replicated bias tile bv has 1.0 there, so the "ones column" that
    makes the PV matmul accumulate softmax denominators (and b_v itself) ride the DVE
    PSUM->SBUF drain as a tensor_add -- no bias matmuls at all.
Device pipeline per core:
  QKV^T matmuls -> S^T = K Q^T per key-chunk with causal column trimming (two K=64
  row-tiled matmuls per chunk, one per head, into the two banks of one PSUM tile) ->
  one ScalarE Exp(scale=1/8) per chunk covering both heads [2,512], PSUM->SBUF fp16 =
  P^T -> causal tri-mask multiply on the 128x128 diagonal blocks only (DVE) -> PV
  matmuls accumulate [q, 64 cols + denominator] per q-block (both heads packed in one
  PSUM bank) -> reciprocal (DVE) * scale (DVE) epilogue -> DMA out per (pair, q-block).
Scheduling: blocks run PAIR-MAJOR (all 4 stripes of head-pair 0, then pair 1, ...).
t-major order skews the workload (early stripes are exp-light -> TensorE-bound first
half, ScalarE-bound endgame); pair-major gives every phase ~1/4 of the exp work against
~1/4 of the matmul work so both engines stay loaded throughout. QKV tiles are a
demand-ordered filler queue drained between S^T chunks (next block's tiles prefetched
via hooks, one V or PV pop per hook); each stripe's PV is deferred into the pair's next
block (inlined per-diagonal-chunk in the final block). V half-0 is produced during
pair-0's phase, half-1 prefetched during pair-1's. Input DMA: few big repacked
transfers on the sync ring ordered by first consumption (wqk pair-0, x stripes 0-1, wv,
x 2-3); constants + remaining wqk pairs ride the scalar HWDGE ring in parallel.
"""

import numpy as np

import concourse.bass as bass
import concourse.tile as tile
from concourse import bacc, mybir
from concourse import bass_utils

F16 = mybir.dt.float16
F32 = mybir.dt.float32

B, N, D = 4, 2048, 1024
H = 16  # global heads
HD = 64
HL = 8  # heads per core
N_CORES = 8
P = 128
NT = N // P  # 16 token tiles
KC = D // P  # 8 contraction chunks
VW = HL * (HD + 1)  # 520
VH = VW // 2  # 260

_cache = {}


def _build():
    nc = bacc.Bacc("TRN2", target_bir_lowering=False, debug=False)

    x_d = nc.dram_tensor("x", [P, 4, KC, 512], F16, kind="ExternalInput").ap()
    wqk_d = nc.dram_tensor("wqk", [P, 4, KC, 256], F16, kind="ExternalInput").ap()
    wv_d = nc.dram_tensor("wv", [P, KC, VW], F16, kind="ExternalInput").ap()
    bqk_d = nc.dram_tensor("bqk", [P, 8], F32, kind="ExternalInput").ap()
    bv_d = nc.dram_tensor("bv", [P, VW], F16, kind="ExternalInput").ap()
    tri_d = nc.dram_tensor("tri", [P, P], F16, kind="ExternalInput").ap()
    out_d = nc.dram_tensor("out", [N, HL * HD], F32, kind="ExternalOutput").ap()

    with tile.TileContext(nc) as tc:
        with (
            tc.tile_pool(name="const", bufs=1) as cpool,
            tc.tile_pool(name="pt", bufs=2) as ptpool,
            tc.tile_pool(name="opair", bufs=3) as oppool,
            tc.tile_pool(name="misc", bufs=6) as mpool,
            tc.tile_pool(name="ps_mm", bufs=2, space="PSUM") as ps_mm,
            tc.tile_pool(name="ps_s", bufs=2, space="PSUM") as ps_s,
            tc.tile_pool(name="ps_o", bufs=2, space="PSUM") as ps_o,
        ):
            # ---- constants / inputs to SBUF ----
            x_sb = [cpool.tile([P, KC, 512], F16, name=f"x{tt}_sb") for tt in range(4)]
            wqk0_sb = cpool.tile([P, KC, 256], F16, name="wqk0_sb")  # pair 0
            wqkR_sb = cpool.tile([P, KC, 768], F16, name="wqkR_sb")  # pairs 1-3
            wv_sb = cpool.tile([P, KC, VW], F16, name="wv_sb")
            bqk_sb = cpool.tile([P, 8], F32, name="bqk_sb")
            bv_sb = cpool.tile([P, VW], F16, name="bv_sb")  # b_v (+ones col) replicated
            tri_sb = cpool.tile([P, P], F16, name="tri_sb")
            qt_sb = cpool.tile([P, 4, N], F16, name="qt_sb")  # Q^T pair-stacked
            kt_sb = cpool.tile([P, 4, N], F16, name="kt_sb")  # K^T pair-stacked
            v_sb = cpool.tile([P, NT, VW], F16, name="v_sb")
            # P^T staging, STATIC per stripe (plane = 2*chunk+hh), reused across
            # pairs: pair p's stripe-t chains drain during pair p's own phase, a full
            # phase before pair p+1 rewrites pt_t -- so the deferred PV chains get a
            # whole phase of slack instead of a 2-block pool deadline.
            pt_t = [
                cpool.tile([P, 2 * (4 * t + 4), 512], F16, name=f"pt{t}_sb")
                for t in range(4)
            ]

            # Preload the exp table set during the DMA fill, so the first real
            # softmax exp doesn't pay ACT_TABLE_LOAD.
            warm = mpool.tile([1, 8], F32, tag="warm", name="warm")
            nc.vector.memset(warm[:], 0.0)
            nc.scalar.activation(warm[:], warm[:], mybir.ActivationFunctionType.Exp)
            # Input DMA ordered by first consumption under the pair-major schedule.
            nc.sync.dma_start(wqk0_sb[:], wqk_d[:, 0, :, :])  # 0.5 MB
            nc.sync.dma_start(x_sb[0][:], x_d[:, 0, :, :])  # 1 MB
            nc.sync.dma_start(x_sb[1][:], x_d[:, 1, :, :])
            nc.sync.dma_start(wv_sb[:], wv_d)  # 1 MB
            nc.sync.dma_start(x_sb[2][:], x_d[:, 2, :, :])
            nc.sync.dma_start(x_sb[3][:], x_d[:, 3, :, :])
            nc.scalar.dma_start(bqk_sb[:], bqk_d)
            nc.scalar.dma_start(tri_sb[:], tri_d)
            nc.scalar.dma_start(bv_sb[:], bv_d)
            nc.scalar.dma_start(wqkR_sb[:, :, 0:256], wqk_d[:, 1, :, :])
            nc.scalar.dma_start(wqkR_sb[:, :, 256:512], wqk_d[:, 2, :, :])
            nc.scalar.dma_start(wqkR_sb[:, :, 512:768], wqk_d[:, 3, :, :])

            done_qk = set()
            done_v = set()

            def wqk_slice(c, k):
                pr = c % 4
                off = 0 if c < 4 else 128
                if pr == 0:
                    return wqk0_sb[:, k, off : off + P]
                base = 256 * (pr - 1) + off
                return wqkR_sb[:, k, base : base + P]

            def emit_qk(c, tt):
                """QKV^T matmul tile for col-chunk c, token stripe tt."""
                if (c, tt) in done_qk:
                    return
                done_qk.add((c, tt))
                pr = c % 4
                pq = ps_mm.tile([P, 512], F32, tag="mm", name=f"pq_{c}_{tt}")
                for k in range(KC):
                    nc.tensor.matmul(
                        pq[:],
                        lhsT=wqk_slice(c, k),
                        rhs=x_sb[tt][:, k, :],
                        start=(k == 0),
                        stop=(k == KC - 1),
                    )
                dst = qt_sb if c < 4 else kt_sb
                nc.vector.tensor_scalar_add(
                    dst[:, pr, tt * 512 : (tt + 1) * 512], pq[:], bqk_sb[:, c : c + 1]
                )

            def emit_v(j, half):
                """V (augmented) for token tile j, half (260 cols each)."""
                if (j, half) in done_v:
                    return
                done_v.add((j, half))
                pv = ps_mm.tile([P, VH], F32, tag="mm", name=f"pv_{j}_{half}")
                for k in range(KC):
                    nc.tensor.matmul(
                        pv[:],
                        lhsT=x_sb[j // 4][:, k, (j % 4) * P : (j % 4 + 1) * P],
                        rhs=wv_sb[:, k, half * VH : (half + 1) * VH],
                        start=(k == 0),
                        stop=(k == KC - 1),
                    )
                # bias (and the denominator ones-column) ride the PSUM->SBUF drain
                nc.vector.tensor_add(
                    v_sb[:, j, half * VH : (half + 1) * VH],
                    pv[:],
                    bv_sb[:, half * VH : (half + 1) * VH],
                )

            state = {}

            def emit_pv_half(p, t, pt, r, hh, ctx):
                """One head's PV chain for q-block i = 4t+r; epilogue+DMA after hh=1.
                Both heads' accumulators share one PSUM bank ([128, 2, 65])."""
                i = 4 * t + r
                if hh == 0:
                    ctx["opair"] = oppool.tile([P, P], F32, tag="op", name=f"op_{p}_{i}")
                    ctx["po"] = po = ps_o.tile([P, 2, 65], F32, tag="o", name=f"po_{p}_{i}")
                else:
                    po = ctx["po"]
                for j in range(i + 1):
                    nc.tensor.matmul(
                        po[:, hh, :],
                        lhsT=pt[:, 2 * j + hh, r * P : (r + 1) * P],
                        rhs=v_sb[:, j, 65 * (2 * p + hh) : 65 * (2 * p + hh) + 65],
                        start=(j == 0),
                        stop=(j == i),
                    )
                if hh == 0:
                    return
                opair = ctx["opair"]
                rc = mpool.tile([P, 2], F32, tag="rc", name=f"rc_{p}_{i}")
                nc.vector.reciprocal(rc[:], po[:, :, 64])
                for h2 in (0, 1):
                    nc.vector.tensor_scalar_mul(
                        opair[:, 64 * h2 : 64 * h2 + 64], po[:, h2, 0:64], rc[:, h2 : h2 + 1]
                    )
                nc.sync.dma_start(out_d[i * P : (i + 1) * P, p * P : (p + 1) * P], opair[:])

            def emit_pv(p, t, pt, r):
                ctx = {}
                emit_pv_half(p, t, pt, r, 0, ctx)
                emit_pv_half(p, t, pt, r, 1, ctx)

            # Prologue: just the first QK tiles so S^T (pair 0, stripe 0) starts ASAP.
            emit_qk(0, 0)
            emit_qk(4, 0)

            pv_queue = []
            # PAIR-MAJOR: all four stripes of pair 0, then pair 1, ...
            blocks = [(p, t) for p in range(4) for t in range(4)]
            for n, (p, t) in enumerate(blocks):
                    last = n == len(blocks) - 1
                    for tt in range(t + 1):
                        emit_qk(p, tt)
                        emit_qk(4 + p, tt)
                    # guard: previous pair's stripe-t chains read pt_t -- finish them
                    # before this block's exps rewrite it (normally already drained
                    # by the one-pop-per-hook pacing)
                    for ch in [c for c in pv_queue if c[1] == t]:
                        pv_queue.remove(ch)
                        emit_pv(*ch)
                    # tiles the NEXT block's S^T will need, prefetched via hooks
                    nxt_qk = []
                    if n + 1 < len(blocks):
                        pn, tn = blocks[n + 1]
                        nxt_qk = [
                            (c, tt)
                            for tt in range(tn + 1)
                            for c in (pn, 4 + pn)
                            if (c, tt) not in done_qk
                        ]
                    # pt layout: [128, plane=2*chunk+hh, 512]
                    pt = pt_t[t]
                    # V tiles: pair 0's phase produces half 0 (its own PV demand),
                    # pair 1's phase prefetches half 1 for pairs 2-3.
                    if p == 0:
                        vpend = [(j, 0) for j in range(4 * t, 4 * t + 4) if (j, 0) not in done_v]
                    elif p == 1:
                        vpend = [(j, 1) for j in range(4 * t, 4 * t + 4) if (j, 1) not in done_v]
                    else:
                        vpend = [
                            (j, p // 2)
                            for j in range(4 * t, 4 * t + 4)
                            if (j, p // 2) not in done_v
                        ]

                    def chunk_hooks(vpend=vpend, nxt_qk=nxt_qk):
                        if pv_queue:
                            emit_pv(*pv_queue.pop(0))
                        elif vpend:
                            emit_v(*vpend.pop(0))
                        if nxt_qk:
                            emit_qk(*nxt_qk.pop(0))

                    # S^T + exp per key-chunk: the pair's two heads run as K=64
                    # row-tiled matmuls (array row-groups 0-1 / 2-3, adjacent PSUM
                    # banks), then one Exp covers both heads. Diagonal chunks only
                    # compute/exp the causal-valid column suffix.
                    for j in range(4 * t + 4):
                        psC = ps_s.tile([P, 2, 512], F32, tag="s", name=f"ps_{p}_{t}_{j}")
                        q0 = 128 * (j - 4 * t) if j >= 4 * t else 0
                        for hh in (0, 1):
                            nc.tensor.matmul(
                                psC[:, hh, q0:512],
                                lhsT=kt_sb[64 * hh : 64 * hh + 64, p, j * P : (j + 1) * P],
                                rhs=qt_sb[
                                    64 * hh : 64 * hh + 64,
                                    p,
                                    t * 512 + q0 : (t + 1) * 512,
                                ],
                                start=True,
                                stop=True,
                            )
                        nc.scalar.activation(
                            pt[:, 2 * j : 2 * j + 2, q0:512],
                            psC[:, :, q0:512],
                            mybir.ActivationFunctionType.Exp,
                            scale=0.125,
                        )
                        chunk_hooks()
                        if last and j >= 4 * t:
                            # final block: mask + PV inline per diagonal chunk so the
                            # tail doesn't serialize after the last exp
                            r = j - 4 * t
                            for hh in (0, 1):
                                blk = pt[:, 2 * j + hh, r * P : (r + 1) * P]
                                nc.vector.tensor_mul(blk, blk, tri_sb[:])
                            emit_pv(p, t, pt, r)
                    if last:
                        continue
                    # causal mask on diagonal 128x128 blocks (DVE: fast and it has
                    # slack; next block's PV pops need these early)
                    for hh in (0, 1):
                        for r in range(4):
                            j = 4 * t + r
                            blk = pt[:, 2 * j + hh, r * P : (r + 1) * P]
                            nc.vector.tensor_mul(blk, blk, tri_sb[:])
                    # V tiles this stripe's PV will need (PV runs during the next
                    # block; guard: force any stragglers now)
                    for j in range(4 * t + 4):
                        emit_v(j, p // 2)
                    pv_queue = [(p, t, pt, r) for r in range(4)]
            while pv_queue:
                emit_pv(*pv_queue.pop(0))
            # flush any unprefetched QKV (normally none)
            for tt in range(4):
                for c in range(8):
                    emit_qk(c, tt)
                for j in range(4 * tt, 4 * tt + 4):
                    emit_v(j, 0)
                    emit_v(j, 1)

    nc.compile()
    return nc


def get_nc():
    if "nc" not in _cache:
        _cache["nc"] = _build()
    return _cache["nc"]


def _prep_core_inputs(x, W, b, bi, hg):
    h0 = hg * HL
    Wq = W[:, 0:D].reshape(D, H, HD)
    Wk = W[:, D : 2 * D].reshape(D, H, HD)
    Wv = W[:, 2 * D :].reshape(D, H, HD)
    bq = b[0:D].reshape(H, HD)
    bk = b[D : 2 * D].reshape(H, HD)
    bv = b[2 * D :].reshape(H, HD)

    # pair-major: pair p occupies cols [256p, 256p+256) as [Q pair | K pair]
    wqk = np.empty((D, 1024), np.float32)
    bqk = np.empty((P, 8), np.float32)
    for c in range(4):
        for half in range(2):
            h = h0 + 2 * c + half
            sl = slice(256 * c + half * HD, 256 * c + half * HD + HD)
            wqk[:, sl] = Wq[:, h]
            bqk[half * HD : (half + 1) * HD, c] = bq[h]
            sl = slice(256 * c + P + half * HD, 256 * c + P + half * HD + HD)
            wqk[:, sl] = Wk[:, h]
            bqk[half * HD : (half + 1) * HD, 4 + c] = bk[h]

    wv_aug = np.zeros((D, VW), np.float32)
    bv_aug = np.zeros((VW,), np.float32)
    for hl in range(HL):
        wv_aug[:, 65 * hl : 65 * hl + HD] = Wv[:, h0 + hl]
        bv_aug[65 * hl : 65 * hl + HD] = bv[h0 + hl]
        bv_aug[65 * hl + HD] = 1.0

    tri = np.triu(np.ones((P, P), np.float32))  # tri[k, q] = 1 where q >= k

    # Repack for contiguous-per-partition DMA runs:
    xT = np.ascontiguousarray(x[bi].astype(np.float16).T)
    xh = np.ascontiguousarray(xT.reshape(KC, P, 4, 512).transpose(1, 2, 0, 3))
    wqkh = np.ascontiguousarray(
        wqk.astype(np.float16).reshape(KC, P, 4, 256).transpose(1, 2, 0, 3)
    )
    wvh = np.ascontiguousarray(
        wv_aug.astype(np.float16).reshape(KC, P, VW).transpose(1, 0, 2)
    )

    return {
        "x": xh,
        "wqk": wqkh,
        "wv": wvh,
        "bqk": bqk,
        "bv": np.broadcast_to(bv_aug.astype(np.float16), (P, VW)).copy(),
        "tri": tri.astype(np.float16),
    }


def make_in_maps(x, W_qkv, b_qkv):
    x = np.asarray(x, dtype=np.float32)
    W = np.asarray(W_qkv, dtype=np.float32)
    b = np.asarray(b_qkv, dtype=np.float32)
    return [_prep_core_inputs(x, W, b, i // 2, i % 2) for i in range(N_CORES)]


def assemble(results):
    out = np.empty((B, N, D), np.float32)
    for i in range(N_CORES):
        bi, hg = i // 2, i % 2
        out[bi, :, hg * 512 : (hg + 1) * 512] = results[i]["out"]
    return out


def run(x, W_qkv, b_qkv, trace=False, tmpdir=None):
    nc = get_nc()
    in_maps = make_in_maps(x, W_qkv, b_qkv)
    res = bass_utils.run_bass_kernel_spmd(
        nc, in_maps, core_ids=list(range(N_CORES)), trace=trace, tmpdir=tmpdir
    )
    return assemble(res.results), res


def kernel(x, W_qkv, b_qkv):
    out, _ = run(x, W_qkv, b_qkv)
    return out


# revision 25
# speedup vs baseline: 1.2509x; 1.0061x over previous
"""Causal multi-head attention (QKV projection + softmax(QK^T)V) on 8 TRN2 NeuronCores.

Problem: x[4,2048,1024] @ W_qkv[1024,3072] + b_qkv -> 16-head causal attention -> [4,2048,1024].

Sharding: core i = (batch bi=i//2, head-group hg=i%2). Each core handles 1 batch x 8 heads,
fully data/tensor-parallel (no collectives). Host pre-arranges per-core inputs (all matmul
operands fp16; accumulation f32 in PSUM), repacked so every DMA reads multi-KB contiguous
runs per partition:
  - x^T as [128, stripe, k-chunk, 512]: one DMA per 512-token stripe, 8KB/partition
    contiguous; per-stripe SBUF tiles keep DMA-completion deps fine-grained.
  - wqk pair-major as [128, pair, k-chunk, 256] (pair p: Q cols then K cols, head-PAIR
    stacked 64+64 rows): pair-0 its own tile/DMA so the first QK matmul waits on 0.5 MB.
  - wv [128, k-chunk, 520]: V columns with per-head stride 65; col 65h+64 is a zero
    column, and the replicated bias tile bv has 1.0 there, so the "ones column" that
    makes the PV matmul accumulate softmax denominators (and b_v itself) ride the DVE
    PSUM->SBUF drain as a tensor_add -- no bias matmuls at all.
Device pipeline per core:
  QKV^T matmuls -> S^T = K Q^T per key-chunk with causal column trimming (two K=64
  row-tiled matmuls per chunk, one per head, into the two banks of one PSUM tile) ->
  one ScalarE Exp(scale=1/8) per chunk covering both heads [2,512], PSUM->SBUF fp16 =
  P^T -> causal tri-mask multiply on the 128x128 diagonal blocks only (DVE) -> PV
  matmuls accumulate [q, 64 cols + denominator] per q-block (both heads packed in one
  PSUM bank) -> reciprocal (DVE) * scale (DVE) epilogue -> DMA out per (pair, q-block).
Scheduling: blocks run PAIR-MAJOR (all 4 stripes of head-pair 0, then pair 1, ...).
t-major order skews the workload (early stripes are exp-light -> TensorE-bound first
half, ScalarE-bound endgame); pair-major gives every phase ~1/4 of the exp work against
~1/4 of the matmul work so both engines stay loaded throughout. QKV tiles are a
demand-ordered filler queue drained between S^T chunks (next block's tiles prefetched
via hooks, one V or PV pop per hook); each stripe's PV is deferred into the pair's next
block (inlined per-diagonal-chunk in the final block). V half-0 is produced during
pair-0's phase, half-1 prefetched during pair-1's. Input DMA: few big repacked
transfers on the sync ring ordered by first consumption (wqk pair-0, x stripes 0-1, wv,
x 2-3); constants + remaining wqk pairs ride the scalar HWDGE ring in parallel.
"""

import numpy as np

import concourse.bass as bass
import concourse.tile as tile
from concourse import bacc, mybir
from concourse import bass_utils

F16 = mybir.dt.float16
F32 = mybir.dt.float32

B, N, D = 4, 2048, 1024
H = 16  # global heads
HD = 64
HL = 8  # heads per core
N_CORES = 8
P = 128
NT = N // P  # 16 token tiles
KC = D // P  # 8 contraction chunks
VW = HL * (HD + 1)  # 520
VH = VW // 2  # 260

_cache = {}


def _build():
    nc = bacc.Bacc("TRN2", target_bir_lowering=False, debug=False)

    x_d = nc.dram_tensor("x", [P, 4, KC, 512], F16, kind="ExternalInput").ap()
    wqk_d = nc.dram_tensor("wqk", [P, 4, KC, 256], F16, kind="ExternalInput").ap()
    wv_d = nc.dram_tensor("wv", [P, 2, KC, VH], F16, kind="ExternalInput").ap()
    bqk_d = nc.dram_tensor("bqk", [P, 8], F32, kind="ExternalInput").ap()
    bv_d = nc.dram_tensor("bv", [P, VW], F16, kind="ExternalInput").ap()
    tri_d = nc.dram_tensor("tri", [P, P], F16, kind="ExternalInput").ap()
    out_d = nc.dram_tensor("out", [N, HL * HD], F32, kind="ExternalOutput").ap()

    with tile.TileContext(nc) as tc:
        with (
            tc.tile_pool(name="const", bufs=1) as cpool,
            tc.tile_pool(name="pt", bufs=2) as ptpool,
            tc.tile_pool(name="opair", bufs=3) as oppool,
            tc.tile_pool(name="misc", bufs=6) as mpool,
            tc.tile_pool(name="ps_mm", bufs=2, space="PSUM") as ps_mm,
            tc.tile_pool(name="ps_s", bufs=2, space="PSUM") as ps_s,
            tc.tile_pool(name="ps_o", bufs=2, space="PSUM") as ps_o,
        ):
            # ---- constants / inputs to SBUF ----
            x_sb = [cpool.tile([P, KC, 512], F16, name=f"x{tt}_sb") for tt in range(4)]
            wqk0_sb = cpool.tile([P, KC, 256], F16, name="wqk0_sb")  # pair 0
            wqkR_sb = cpool.tile([P, KC, 768], F16, name="wqkR_sb")  # pairs 1-3
            wv_sb = cpool.tile([P, KC, VW], F16, name="wv_sb")
            bqk_sb = cpool.tile([P, 8], F32, name="bqk_sb")
            bv_sb = cpool.tile([P, VW], F16, name="bv_sb")  # b_v (+ones col) replicated
            tri_sb = cpool.tile([P, P], F16, name="tri_sb")
            qt_sb = cpool.tile([P, 4, N], F16, name="qt_sb")  # Q^T pair-stacked
            kt_sb = cpool.tile([P, 4, N], F16, name="kt_sb")  # K^T pair-stacked
            v_sb = cpool.tile([P, NT, VW], F16, name="v_sb")
            # P^T staging, STATIC per stripe (plane = 2*chunk+hh), reused across
            # pairs: pair p's stripe-t chains drain during pair p's own phase, a full
            # phase before pair p+1 rewrites pt_t -- so the deferred PV chains get a
            # whole phase of slack instead of a 2-block pool deadline.
            pt_t = [
                cpool.tile([P, 2 * (4 * t + 4), 512], F16, name=f"pt{t}_sb")
                for t in range(4)
            ]

            # Preload the exp table set during the DMA fill, so the first real
            # softmax exp doesn't pay ACT_TABLE_LOAD.
            warm = mpool.tile([1, 8], F32, tag="warm", name="warm")
            nc.vector.memset(warm[:], 0.0)
            nc.scalar.activation(warm[:], warm[:], mybir.ActivationFunctionType.Exp)
            # Input DMA ordered by first consumption under the pair-major schedule.
            nc.sync.dma_start(wqk0_sb[:], wqk_d[:, 0, :, :])  # 0.5 MB
            nc.sync.dma_start(x_sb[0][:], x_d[:, 0, :, :])  # 1 MB
            nc.sync.dma_start(wv_sb[:, :, 0:VH], wv_d[:, 0, :, :])  # V half-0
            nc.sync.dma_start(x_sb[1][:], x_d[:, 1, :, :])
            nc.sync.dma_start(wv_sb[:, :, VH:VW], wv_d[:, 1, :, :])  # V half-1
            nc.sync.dma_start(x_sb[2][:], x_d[:, 2, :, :])
            nc.sync.dma_start(x_sb[3][:], x_d[:, 3, :, :])
            nc.scalar.dma_start(bqk_sb[:], bqk_d)
            nc.scalar.dma_start(tri_sb[:], tri_d)
            nc.scalar.dma_start(bv_sb[:], bv_d)
            nc.scalar.dma_start(wqkR_sb[:, :, 0:256], wqk_d[:, 1, :, :])
            nc.scalar.dma_start(wqkR_sb[:, :, 256:512], wqk_d[:, 2, :, :])
            nc.scalar.dma_start(wqkR_sb[:, :, 512:768], wqk_d[:, 3, :, :])

            done_qk = set()
            done_v = set()

            def wqk_slice(c, k):
                pr = c % 4
                off = 0 if c < 4 else 128
                if pr == 0:
                    return wqk0_sb[:, k, off : off + P]
                base = 256 * (pr - 1) + off
                return wqkR_sb[:, k, base : base + P]

            def emit_qk(c, tt):
                """QKV^T matmul tile for col-chunk c, token stripe tt."""
                if (c, tt) in done_qk:
                    return
                done_qk.add((c, tt))
                pr = c % 4
                pq = ps_mm.tile([P, 512], F32, tag="mm", name=f"pq_{c}_{tt}")
                for k in range(KC):
                    nc.tensor.matmul(
                        pq[:],
                        lhsT=wqk_slice(c, k),
                        rhs=x_sb[tt][:, k, :],
                        start=(k == 0),
                        stop=(k == KC - 1),
                    )
                dst = qt_sb if c < 4 else kt_sb
                nc.vector.tensor_scalar_add(
                    dst[:, pr, tt * 512 : (tt + 1) * 512], pq[:], bqk_sb[:, c : c + 1]
                )

            def emit_v(j, half):
                """V (augmented) for token tile j, half (260 cols each)."""
                if (j, half) in done_v:
                    return
                done_v.add((j, half))
                pv = ps_mm.tile([P, VH], F32, tag="mm", name=f"pv_{j}_{half}")
                for k in range(KC):
                    nc.tensor.matmul(
                        pv[:],
                        lhsT=x_sb[j // 4][:, k, (j % 4) * P : (j % 4 + 1) * P],
                        rhs=wv_sb[:, k, half * VH : (half + 1) * VH],
                        start=(k == 0),
                        stop=(k == KC - 1),
                    )
                # bias (and the denominator ones-column) ride the PSUM->SBUF drain
                nc.vector.tensor_add(
                    v_sb[:, j, half * VH : (half + 1) * VH],
                    pv[:],
                    bv_sb[:, half * VH : (half + 1) * VH],
                )

            state = {}

            def emit_pv_half(p, t, pt, r, hh, ctx):
                """One head's PV chain for q-block i = 4t+r; epilogue+DMA after hh=1.
                Both heads' accumulators share one PSUM bank ([128, 2, 65])."""
                i = 4 * t + r
                if hh == 0:
                    ctx["opair"] = oppool.tile([P, P], F32, tag="op", name=f"op_{p}_{i}")
                    ctx["po"] = po = ps_o.tile([P, 2, 65], F32, tag="o", name=f"po_{p}_{i}")
                else:
                    po = ctx["po"]
                for j in range(i + 1):
                    nc.tensor.matmul(
                        po[:, hh, :],
                        lhsT=pt[:, 2 * j + hh, r * P : (r + 1) * P],
                        rhs=v_sb[:, j, 65 * (2 * p + hh) : 65 * (2 * p + hh) + 65],
                        start=(j == 0),
                        stop=(j == i),
                    )
                if hh == 0:
                    return
                opair = ctx["opair"]
                rc = mpool.tile([P, 2], F32, tag="rc", name=f"rc_{p}_{i}")
                nc.vector.reciprocal(rc[:], po[:, :, 64])
                for h2 in (0, 1):
                    nc.vector.tensor_scalar_mul(
                        opair[:, 64 * h2 : 64 * h2 + 64], po[:, h2, 0:64], rc[:, h2 : h2 + 1]
                    )
                nc.sync.dma_start(out_d[i * P : (i + 1) * P, p * P : (p + 1) * P], opair[:])

            def emit_pv(p, t, pt, r):
                ctx = {}
                emit_pv_half(p, t, pt, r, 0, ctx)
                emit_pv_half(p, t, pt, r, 1, ctx)

            # Prologue: just the first QK tiles so S^T (pair 0, stripe 0) starts ASAP.
            emit_qk(0, 0)
            emit_qk(4, 0)

            pv_queue = []
            # PAIR-MAJOR: all four stripes of pair 0, then pair 1, ...
            blocks = [(p, t) for p in range(4) for t in range(4)]
            for n, (p, t) in enumerate(blocks):
                    last = n == len(blocks) - 1
                    for tt in range(t + 1):
                        emit_qk(p, tt)
                        emit_qk(4 + p, tt)
                    # guard: previous pair's stripe-t chains read pt_t -- finish them
                    # before this block's exps rewrite it (normally already drained
                    # by the one-pop-per-hook pacing)
                    for ch in [c for c in pv_queue if c[1] == t]:
                        pv_queue.remove(ch)
                        emit_pv(*ch)
                    # tiles the NEXT block's S^T will need, prefetched via hooks
                    nxt_qk = []
                    if n + 1 < len(blocks):
                        pn, tn = blocks[n + 1]
                        nxt_qk = [
                            (c, tt)
                            for tt in range(tn + 1)
                            for c in (pn, 4 + pn)
                            if (c, tt) not in done_qk
                        ]
                    # pt layout: [128, plane=2*chunk+hh, 512]
                    pt = pt_t[t]
                    # V tiles: pair 0's phase produces half 0 (its own PV demand),
                    # pair 1's phase prefetches half 1 for pairs 2-3.
                    if p == 0:
                        vpend = [(j, 0) for j in range(4 * t, 4 * t + 4) if (j, 0) not in done_v]
                    elif p == 1:
                        vpend = [(j, 1) for j in range(4 * t, 4 * t + 4) if (j, 1) not in done_v]
                    else:
                        vpend = [
                            (j, p // 2)
                            for j in range(4 * t, 4 * t + 4)
                            if (j, p // 2) not in done_v
                        ]

                    def chunk_hooks(vpend=vpend, nxt_qk=nxt_qk):
                        if pv_queue:
                            emit_pv(*pv_queue.pop(0))
                        elif vpend:
                            emit_v(*vpend.pop(0))
                        if nxt_qk:
                            emit_qk(*nxt_qk.pop(0))

                    # S^T + exp per key-chunk: the pair's two heads run as K=64
                    # row-tiled matmuls (array row-groups 0-1 / 2-3, adjacent PSUM
                    # banks), then one Exp covers both heads. Diagonal chunks only
                    # compute/exp the causal-valid column suffix.
                    for j in range(4 * t + 4):
                        psC = ps_s.tile([P, 2, 512], F32, tag="s", name=f"ps_{p}_{t}_{j}")
                        q0 = 128 * (j - 4 * t) if j >= 4 * t else 0
                        for hh in (0, 1):
                            nc.tensor.matmul(
                                psC[:, hh, q0:512],
                                lhsT=kt_sb[64 * hh : 64 * hh + 64, p, j * P : (j + 1) * P],
                                rhs=qt_sb[
                                    64 * hh : 64 * hh + 64,
                                    p,
                                    t * 512 + q0 : (t + 1) * 512,
                                ],
                                start=True,
                                stop=True,
                            )
                        nc.scalar.activation(
                            pt[:, 2 * j : 2 * j + 2, q0:512],
                            psC[:, :, q0:512],
                            mybir.ActivationFunctionType.Exp,
                            scale=0.125,
                        )
                        chunk_hooks()
                        if last and j >= 4 * t:
                            # final block: mask + PV inline per diagonal chunk so the
                            # tail doesn't serialize after the last exp
                            r = j - 4 * t
                            for hh in (0, 1):
                                blk = pt[:, 2 * j + hh, r * P : (r + 1) * P]
                                nc.vector.tensor_mul(blk, blk, tri_sb[:])
                            emit_pv(p, t, pt, r)
                    if last:
                        continue
                    # causal mask on diagonal 128x128 blocks (DVE: fast and it has
                    # slack; next block's PV pops need these early)
                    for hh in (0, 1):
                        for r in range(4):
                            j = 4 * t + r
                            blk = pt[:, 2 * j + hh, r * P : (r + 1) * P]
                            nc.vector.tensor_mul(blk, blk, tri_sb[:])
                    # V tiles this stripe's PV will need (PV runs during the next
                    # block; guard: force any stragglers now)
                    for j in range(4 * t + 4):
                        emit_v(j, p // 2)
                    pv_queue = [(p, t, pt, r) for r in range(4)]
            while pv_queue:
                emit_pv(*pv_queue.pop(0))
            # flush any unprefetched QKV (normally none)
            for tt in range(4):
                for c in range(8):
                    emit_qk(c, tt)
                for j in range(4 * tt, 4 * tt + 4):
                    emit_v(j, 0)
                    emit_v(j, 1)

    nc.compile()
    return nc


def get_nc():
    if "nc" not in _cache:
        _cache["nc"] = _build()
    return _cache["nc"]


def _prep_core_inputs(x, W, b, bi, hg):
    h0 = hg * HL
    Wq = W[:, 0:D].reshape(D, H, HD)
    Wk = W[:, D : 2 * D].reshape(D, H, HD)
    Wv = W[:, 2 * D :].reshape(D, H, HD)
    bq = b[0:D].reshape(H, HD)
    bk = b[D : 2 * D].reshape(H, HD)
    bv = b[2 * D :].reshape(H, HD)

    # pair-major: pair p occupies cols [256p, 256p+256) as [Q pair | K pair]
    wqk = np.empty((D, 1024), np.float32)
    bqk = np.empty((P, 8), np.float32)
    for c in range(4):
        for half in range(2):
            h = h0 + 2 * c + half
            sl = slice(256 * c + half * HD, 256 * c + half * HD + HD)
            wqk[:, sl] = Wq[:, h]
            bqk[half * HD : (half + 1) * HD, c] = bq[h]
            sl = slice(256 * c + P + half * HD, 256 * c + P + half * HD + HD)
            wqk[:, sl] = Wk[:, h]
            bqk[half * HD : (half + 1) * HD, 4 + c] = bk[h]

    wv_aug = np.zeros((D, VW), np.float32)
    bv_aug = np.zeros((VW,), np.float32)
    for hl in range(HL):
        wv_aug[:, 65 * hl : 65 * hl + HD] = Wv[:, h0 + hl]
        bv_aug[65 * hl : 65 * hl + HD] = bv[h0 + hl]
        bv_aug[65 * hl + HD] = 1.0

    tri = np.triu(np.ones((P, P), np.float32))  # tri[k, q] = 1 where q >= k

    # Repack for contiguous-per-partition DMA runs:
    xT = np.ascontiguousarray(x[bi].astype(np.float16).T)
    xh = np.ascontiguousarray(xT.reshape(KC, P, 4, 512).transpose(1, 2, 0, 3))
    wqkh = np.ascontiguousarray(
        wqk.astype(np.float16).reshape(KC, P, 4, 256).transpose(1, 2, 0, 3)
    )
    wvh = np.ascontiguousarray(
        wv_aug.astype(np.float16).reshape(KC, P, 2, VH).transpose(1, 2, 0, 3)
    )

    return {
        "x": xh,
        "wqk": wqkh,
        "wv": wvh,
        "bqk": bqk,
        "bv": np.broadcast_to(bv_aug.astype(np.float16), (P, VW)).copy(),
        "tri": tri.astype(np.float16),
    }


def make_in_maps(x, W_qkv, b_qkv):
    x = np.asarray(x, dtype=np.float32)
    W = np.asarray(W_qkv, dtype=np.float32)
    b = np.asarray(b_qkv, dtype=np.float32)
    return [_prep_core_inputs(x, W, b, i // 2, i % 2) for i in range(N_CORES)]


def assemble(results):
    out = np.empty((B, N, D), np.float32)
    for i in range(N_CORES):
        bi, hg = i // 2, i % 2
        out[bi, :, hg * 512 : (hg + 1) * 512] = results[i]["out"]
    return out


def run(x, W_qkv, b_qkv, trace=False, tmpdir=None):
    nc = get_nc()
    in_maps = make_in_maps(x, W_qkv, b_qkv)
    res = bass_utils.run_bass_kernel_spmd(
        nc, in_maps, core_ids=list(range(N_CORES)), trace=trace, tmpdir=tmpdir
    )
    return assemble(res.results), res


def kernel(x, W_qkv, b_qkv):
    out, _ = run(x, W_qkv, b_qkv)
    return out
